# revision 46
# baseline (speedup 1.0000x reference)
"""nn_CustomAttention on 8 Trainium2 NeuronCores.

Full (unsharded) inputs in, full output out. Internally: data-parallel over
batch (2) x tensor-parallel over heads (16 -> 4 per core), ReduceScatter
(sum over the 4 TP ranks) after the output projection.

Math per batch b (reference):
  qkv = concat(q[b], k[b], v[b]) @ W_qkv.T     # dense over all 3C=3072 inputs
  per head: scores = qh kh^T * hd^-0.5, softmax over keys, x = P @ vh
  out = x @ W_proj.T + b_proj

Device kernel (SPMD, one program for all 8 cores; per-core behavior comes
only from the data each core receives). v3:

  - QKV projection in fp8e4 DoubleRow (half cycles/row, 256-deep
    contraction per matmul) with a same-scale 3-term residual expansion
    x@w ~= x8@w8 + x8@dw8 + dx8@w8 (dw8/dx8 are fp8 of the quantization
    residuals at the SAME scale, living in fp8's subnormal range), which
    restores ~bf16 accuracy at 3/4 of bf16's PE cost. W is pre-scaled x32
    on the host so fp8 hits its normal range; the x32*x32 scores scale is
    folded into the softmax exp's input scale, and the x32 on v into the
    denominator ones-column (=32), so downstream stays at true scale.
  - Attention (scores, exp, AV) and output projection in bf16: fp8 there
    fails the 2e-2 gate (softmax weight quantization error dominates;
    HW-verified). Scores S^T (keys on partitions), softmax denominator =
    ones-row of the AV accumulation, normalize via DVE reciprocal +
    GpSimd partition broadcast.
  - x and W resident in SBUF (fp8 pairs, 132KB/partition total) so the
    projection passes can be scheduled freely: k(s0,s1)+q(s0) accumulate
    cx-major pipelined with the DMA stream, then k(s2,s3); v-projection
    chunks woven into strip 0 pair 0 just before the AV that needs them;
    q(s+1) woven into pair 1 of strip s.
  - bf16 ReduceScatter per strip (the collective has a ~15us constant
    cost; 4 equal strips, the last is the tail), y bf16 (host converts).
"""
import numpy as np
import ml_dtypes

import concourse.bass as bass
import concourse.mybir as mybir
import concourse.tile as tile
from concourse import bacc, bass_utils

B, N, C, H, HD = 2, 2048, 1024, 16, 64
HPC = 4          # heads per core
TP = 4           # tensor-parallel group size
NCORES = 8
SW = 512         # n-strip width
NSTRIPS = N // SW
NJC = N // 128   # key chunks
NC2 = 12         # DoubleRow contraction chunks (3C / 256)
SCALE = HD ** -0.5
F32 = mybir.dt.float32
BF16 = mybir.dt.bfloat16
F8 = mybir.dt.float8e4
DR = mybir.MatmulPerfMode.DoubleRow
ExpF = mybir.ActivationFunctionType.Exp
QKV_FP8 = True    # fp8-DR 3-term residual qkv vs plain bf16 qkv
W_SCALE = 32.0 if QKV_FP8 else 1.0   # host W_qkv scale for fp8 normal range
EXP_BIAS = -3.0   # exp(s-3): softmax-invariant shift, keeps exp moderate

_CACHE = {}
LAST_EXEC_TIME_NS = None


def build_nc(reps=1):
    # reps>1 repeats the whole body (incl. weight DMA) for steady-state
    # benching; the graded path always uses reps=1.
    nc = bacc.Bacc("TRN2", target_bir_lowering=False, debug=False,
                   num_devices=NCORES)
    if QKV_FP8:
        # DoubleRow layouts: chunk c, partition p, tile t -> concat-feature
        # c*256 + t*128 + p
        x8a = nc.dram_tensor("x8a", [NC2, 128, 2, N], F8,
                             kind="ExternalInput").ap()
        dx8a = nc.dram_tensor("dx8a", [NC2, 128, 2, N], F8,
                              kind="ExternalInput").ap()
        w8a = nc.dram_tensor("w8a", [NC2, 128, 2, 768], F8,
                             kind="ExternalInput").ap()
        dw8a = nc.dram_tensor("dw8a", [NC2, 128, 2, 768], F8,
                              kind="ExternalInput").ap()
    else:
        # bf16: chunk c covers concat-features [c*128, (c+1)*128)
        xba = nc.dram_tensor("xba", [2 * NC2, 128, N], BF16,
                             kind="ExternalInput").ap()
        wba = nc.dram_tensor("wba", [2 * NC2, 128, 768], BF16,
                             kind="ExternalInput").ap()
    wproj = nc.dram_tensor("wproj", [2, 128, C], BF16, kind="ExternalInput").ap()
    bias = nc.dram_tensor("bias", [1, C], F32, kind="ExternalInput").ap()
    # y rows: quarter q (128 rows) = this rank's chunk of the RS over
    # n-strip q ([q*512, (q+1)*512)); bf16 — host converts back to f32
    y = nc.dram_tensor("y", [N // TP, C], BF16, kind="ExternalOutput").ap()

    with tile.TileContext(nc) as tc:
      for rep in range(reps):
        with tc.tile_pool(name=f"singles{rep}", bufs=1) as singles, \
             tc.tile_pool(name=f"dram{rep}", bufs=1, space="DRAM") as dram:
            x_sb, dx_sb, w_sb, dw_sb = [], [], [], []
            if QKV_FP8:
                for cx in range(NC2):
                    for lst, srct, nm in ((w_sb, w8a, "w"), (dw_sb, dw8a, "dw"),
                                          (x_sb, x8a, "x"), (dx_sb, dx8a, "dx")):
                        shape = ([128, 2, 768] if nm in ("w", "dw")
                                 else [128, 2, N])
                        t = singles.tile(shape, F8, name=f"{nm}{cx}",
                                         tag=f"{nm}{cx}")
                        nc.sync.dma_start(t[:], srct[cx])
                        lst.append(t)
            else:
                for cx in range(2 * NC2):
                    wt = singles.tile([128, 768], BF16, name=f"w{cx}",
                                      tag=f"w{cx}")
                    nc.sync.dma_start(wt[:], wba[cx])
                    w_sb.append(wt)
                    xt = singles.tile([128, N], BF16, name=f"x{cx}",
                                      tag=f"x{cx}")
                    nc.sync.dma_start(xt[:], xba[cx])
                    x_sb.append(xt)
            wp_tiles = []
            for co in range(2):
                wpt = singles.tile([128, C], BF16, name=f"wp{co}", tag=f"wp{co}")
                nc.sync.dma_start(wpt[:], wproj[co])
                wp_tiles.append(wpt)
            bias_sb = singles.tile([1, C], F32, name="bias_sb")
            nc.sync.dma_start(bias_sb[:], bias)
            bias_bc = singles.tile([128, C], F32, name="bias_bc")
            nc.gpsimd.partition_broadcast(bias_bc[:], bias_sb[:])

            # q,k head-transposed projections (x32 scale): rows = 2 heads
            # x 64d; fc 0,1 = q heads (0,1),(2,3); fc 2,3 = k heads
            qk_sb = singles.tile([128, 4, N], BF16, name="qk_sb")
            # v key-major bf16 (x32) + ones(=32) column per head
            v_sb = singles.tile([128, NJC, HPC, 65], BF16, name="v_sb")
            ones1 = singles.tile([128, 1], F32, name="ones1")
            nc.vector.memset(ones1[:], W_SCALE)
            ebias = singles.tile([128, 1], F32, name="ebias")
            nc.vector.memset(ebias[:], EXP_BIAS)
            nc.vector.tensor_copy(
                v_sb[:, :, :, 64],
                ones1[:, :, None].to_broadcast([128, NJC, HPC]))
            # normalized attention out, feature-major: [ci, co, n]
            oT_sb = singles.tile([128, 2, N], BF16, name="oT_sb")

            cc_in = dram.tile([N, C], BF16, name="cc_in")
            # NOTE: Shared addr_space is only allowed for AllGather/AllReduce
            cc_out = [dram.tile([SW // TP, C], BF16, name=f"cc_out{i}")
                      for i in range(NSTRIPS)]

            # the 3 residual terms: (stationary-w, moving-x) operand pairs
            TERMS = ((w_sb, x_sb), (dw_sb, x_sb), (w_sb, dx_sb))

            # Static PSUM map (8 banks): "s" [128,1024] bufs=2 -> 4 banks
            # (scores, also phase-A accumulators via half-slices), po0/po1
            # -> 2 banks, "aux" [128,512] bufs=2 -> 2 banks (shared in
            # sequence by phase-A, v-proj, q-weave and proj tiles).
            with tc.tile_pool(name="ep", bufs=4) as ep, \
                 tc.tile_pool(name="smp", bufs=2) as smp, \
                 tc.tile_pool(name="outp", bufs=4) as outp, \
                 tc.tile_pool(name="ps_b", bufs=2, space="PSUM") as ps_b:

                def qk_psums(n):
                    """n concurrent accumulation slices [128, SW]: at most
                    two "s" tiles (split in half) + two aux tiles."""
                    assert n <= 6
                    out = []
                    for i in range(min(2, (n + 1) // 2)):
                        t = ps_b.tile([128, 1024], F32, tag="s",
                                      name=f"pbig{i}", bufs=2)
                        out.append(t[:, 0:SW])
                        out.append(t[:, SW:2 * SW])
                    while len(out) < n:
                        out.append(ps_b.tile([128, SW], F32, tag="aux",
                                             name="paux", bufs=2)[:])
                    return out[:n]

                def qk_matmul(pt, fc, s, cx, term):
                    """one matmul: the (cx, term) contribution to the
                    [fc-feature-block x strip-s] accumulator"""
                    if QKV_FP8:
                        wop, xop = TERMS[term]
                        nc.tensor.matmul(
                            pt,
                            wop[cx][:, :, fc * 128:(fc + 1) * 128],
                            xop[cx][:, :, s * SW:(s + 1) * SW],
                            start=(cx == 0 and term == 0),
                            stop=(cx == NC2 - 1 and term == 2),
                            perf_mode=DR)
                    else:
                        nc.tensor.matmul(
                            pt,
                            w_sb[cx][:, fc * 128:(fc + 1) * 128],
                            x_sb[cx][:, s * SW:(s + 1) * SW],
                            start=(cx == 0 and term == 0),
                            stop=(cx == 2 * NC2 - 1 and term == 0))

                def qk_pass(groups):
                    """groups: list of (fc, strip). One PSUM slice each,
                    accumulated cx-major so it pipelines with the x DMA."""
                    pts = qk_psums(len(groups))
                    if QKV_FP8:
                        cxterms = [(cx, t) for cx in range(NC2)
                                   for t in range(3)]
                    else:
                        cxterms = [(cx, 0) for cx in range(2 * NC2)]
                    for cx, term in cxterms:
                        for g, (fc, s) in enumerate(groups):
                            qk_matmul(pts[g], fc, s, cx, term)
                    for g, (fc, s) in enumerate(groups):
                        nc.vector.tensor_copy(
                            qk_sb[:, fc, s * SW:(s + 1) * SW], pts[g])

                def make_q_weave(s):
                    """q-projection for strip s as weave units, 2 fc groups
                    x (12 cx x 3 terms), holding both aux buffers."""
                    pts = [ps_b.tile([128, SW], F32, tag="aux",
                                     name=f"pq{i}", bufs=2)
                           for i in range(2)]

                    def unit(cx, term):
                        def emit():
                            for fc in range(2):
                                qk_matmul(pts[fc][:], fc, s, cx, term)
                            if (cx == NC2 - 1 and term == 2) if QKV_FP8 \
                                    else (cx == 2 * NC2 - 1):
                                for fc in range(2):
                                    nc.vector.tensor_copy(
                                        qk_sb[:, fc, s * SW:(s + 1) * SW],
                                        pts[fc][:])
                        return emit
                    if QKV_FP8:
                        return [unit(cx, term)
                                for cx in range(NC2) for term in range(3)]
                    return [unit(cx, 0) for cx in range(2 * NC2)]

                def make_v_weave():
                    """v-projection chunks: chunk c covers tokens (=keys)
                    [c*128,(c+1)*128). Emitted before AV(jc=c) of strip 0."""
                    def unit(c):
                        def emit():
                            pv = ps_b.tile([128, SW], F32, tag="aux",
                                           name="pv", bufs=2)[:, 0:256]
                            if QKV_FP8:
                                for cx in range(NC2):
                                    for term in range(3):
                                        wop, xop = TERMS[term]
                                        nc.tensor.matmul(
                                            pv,
                                            xop[cx][:, :,
                                                    c * 128:(c + 1) * 128],
                                            wop[cx][:, :, 512:768],
                                            start=(cx == 0 and term == 0),
                                            stop=(cx == NC2 - 1 and term == 2),
                                            perf_mode=DR)
                            else:
                                for cx in range(2 * NC2):
                                    nc.tensor.matmul(
                                        pv,
                                        x_sb[cx][:, c * 128:(c + 1) * 128],
                                        w_sb[cx][:, 512:768],
                                        start=(cx == 0),
                                        stop=(cx == 2 * NC2 - 1))
                            nc.vector.tensor_copy(
                                v_sb[:, c, :, 0:64],
                                pv.rearrange("p (h d) -> p h d", h=HPC))
                        return emit
                    return [unit(c) for c in range(NJC)]

                def make_proj_weave(s):
                    """output projection + bias + RS for strip s as weave
                    units (one per [nch, mh] output tile, then the
                    collective+y-DMA as the final unit)."""
                    def unit(nch, mh):
                        def emit():
                            pp = ps_b.tile([128, SW], F32, tag="aux",
                                           name="pp", bufs=2)
                            for co in range(2):
                                nc.tensor.matmul(
                                    pp[:],
                                    oT_sb[:, co, nch * 128:(nch + 1) * 128],
                                    wp_tiles[co][:, mh * SW:(mh + 1) * SW],
                                    start=(co == 0), stop=(co == 1))
                            ot = outp.tile([128, SW], BF16, tag="ot",
                                           name="ot")
                            nc.vector.tensor_add(
                                ot[:], pp[:],
                                bias_bc[:, mh * SW:(mh + 1) * SW])
                            nc.sync.dma_start(
                                cc_in[nch * 128:(nch + 1) * 128,
                                      mh * SW:(mh + 1) * SW], ot[:])
                        return emit

                    def fin():
                        nc.gpsimd.collective_compute(
                            "ReduceScatter", mybir.AluOpType.add,
                            replica_groups=[[0, 1, 2, 3], [4, 5, 6, 7]],
                            ins=[cc_in[s * SW:(s + 1) * SW, :].opt()],
                            outs=[cc_out[s][:].opt()])
                        nc.sync.dma_start(y[s * 128:(s + 1) * 128, :],
                                          cc_out[s][:])
                    return [unit(nch, mh)
                            for nch in range(4 * s, 4 * (s + 1))
                            for mh in range(2)] + [fin]

                def proj_quarter(s):
                    for u in make_proj_weave(s):
                        u()

                def attention_pair(s, p, weave, weave_first=False):
                    """scores+exp+AV for head pair (2p, 2p+1) of strip s,
                    interleaving `weave` closures into the jc loop."""
                    po = [ps_b.tile([65, SW], F32, tag=f"po{par}",
                                    name=f"po{par}", bufs=1)
                          for par in range(2)]
                    wv = list(weave)
                    for jc in range(NJC):
                        ps2 = ps_b.tile([128, 1024], F32, tag="s",
                                        name="ps2", bufs=2)
                        for par in range(2):
                            hp = par * 64
                            nc.tensor.matmul(
                                ps2[:, par * SW:(par + 1) * SW],
                                qk_sb[hp:hp + 64, 2 + p,
                                      jc * 128:(jc + 1) * 128],
                                qk_sb[hp:hp + 64, p,
                                      s * SW:(s + 1) * SW],
                                start=True, stop=True)
                        et = ep.tile([128, 1024], BF16, tag="e", name="et")
                        # scores psum is (32q)x(32k): exp(psum/1024 - 3)
                        nc.scalar.activation(
                            out=et[:], in_=ps2[:], func=ExpF,
                            bias=ebias[:], scale=1.0 / (W_SCALE * W_SCALE))
                        # weave: spread the units evenly over the jc loop
                        take = len(weave) * (jc + 1) // NJC - \
                            len(weave) * jc // NJC
                        if weave_first:
                            for _ in range(take):
                                if wv:
                                    wv.pop(0)()
                        for par in range(2):
                            h = 2 * p + par
                            nc.tensor.matmul(
                                po[par][:], v_sb[:, jc, h, :],
                                et[:, par * SW:(par + 1) * SW],
                                start=(jc == 0), stop=(jc == NJC - 1))
                        if not weave_first:
                            for _ in range(take):
                                if wv:
                                    wv.pop(0)()
                    while wv:
                        wv.pop(0)()
                    for par in range(2):
                        hp = par * 64
                        recip = smp.tile([1, SW], F32, tag=f"recip{par}",
                                         name="recip")
                        nc.vector.reciprocal(recip[:], po[par][64:65, :])
                        bc = smp.tile([64, SW], F32, tag=f"bc{par}",
                                      name="bc")
                        nc.gpsimd.partition_broadcast(bc[:], recip[:])
                        nc.vector.tensor_mul(
                            oT_sb[hp:hp + 64, p, s * SW:(s + 1) * SW],
                            po[par][0:64, :], bc[:])

                # phase A: k(s0), k(s1), q(s0) pipelined with the DMA
                # stream, then k(s2), k(s3)
                qk_pass([(2, 0), (3, 0), (2, 1), (3, 1), (0, 0), (1, 0)])
                qk_pass([(2, 2), (3, 2), (2, 3), (3, 3)])

                v_weave = make_v_weave()
                for s in range(NSTRIPS):
                    if s == 0:
                        attention_pair(s, 0, v_weave, weave_first=True)
                    else:
                        attention_pair(s, 0, [])
                    q_weave = make_q_weave(s + 1) if s + 1 < NSTRIPS else []
                    attention_pair(s, 1, q_weave)
                    proj_quarter(s)
    nc.compile()
    return nc


def make_in_maps(q, k, v, W_qkv, W_proj, b_proj, **_):
    bf = lambda a: np.ascontiguousarray(a).astype(ml_dtypes.bfloat16)
    F8NP = ml_dtypes.float8_e4m3

    def dr(a, cols):
        # [3072, cols] -> [NC2, 128, 2, cols]: (c,p,t) = feature c*256+t*128+p
        return np.ascontiguousarray(
            a.reshape(NC2, 2, 128, cols).transpose(0, 2, 1, 3))

    in_maps = []
    for core in range(NCORES):
        b, r = divmod(core, TP)
        lo, hi = r * HPC * HD, (r + 1) * HPC * HD    # this core's 256 features
        wq = W_qkv[lo:hi, :] * np.float32(SCALE)
        wk = W_qkv[C + lo:C + hi, :]
        wv = W_qkv[2 * C + lo:2 * C + hi, :]
        wsel = np.concatenate([wq, wk, wv], axis=0)       # [768, 3072]
        wqkvT = np.ascontiguousarray(wsel.T) * np.float32(W_SCALE)  # [3072,768]
        wprojT = np.ascontiguousarray(W_proj[:, lo:hi].T)  # [256, 1024]
        bias = b_proj if r == 0 else np.zeros_like(b_proj)
        xcat = np.concatenate(
            [np.ascontiguousarray(q[b].T), np.ascontiguousarray(k[b].T),
             np.ascontiguousarray(v[b].T)], axis=0)       # [3072, N]
        m = {
            "wproj": bf(wprojT.reshape(2, 128, C)),
            "bias": np.ascontiguousarray(bias[None, :], dtype=np.float32),
        }
        if QKV_FP8:
            w8 = wqkvT.astype(F8NP)
            dw8 = (wqkvT - w8.astype(np.float32)).astype(F8NP)
            x8 = xcat.astype(F8NP)
            dx8 = (xcat - x8.astype(np.float32)).astype(F8NP)
            m.update(x8a=dr(x8, N), dx8a=dr(dx8, N),
                     w8a=dr(w8, 768), dw8a=dr(dw8, 768))
        else:
            m.update(xba=bf(xcat.reshape(2 * NC2, 128, N)),
                     wba=bf(wqkvT.reshape(2 * NC2, 128, 768)))
        in_maps.append(m)
    return in_maps


def kernel(q, k, v, W_qkv, W_proj, b_proj, trace=False):
    global LAST_EXEC_TIME_NS
    q = np.asarray(q, dtype=np.float32)
    k = np.asarray(k, dtype=np.float32)
    v = np.asarray(v, dtype=np.float32)
    W_qkv = np.asarray(W_qkv, dtype=np.float32)
    W_proj = np.asarray(W_proj, dtype=np.float32)
    b_proj = np.asarray(b_proj, dtype=np.float32)

    if "nc" not in _CACHE:
        _CACHE["nc"] = build_nc()
    nc = _CACHE["nc"]
    in_maps = make_in_maps(q, k, v, W_qkv, W_proj, b_proj)
    res = bass_utils.run_bass_kernel_spmd(
        nc, in_maps, core_ids=list(range(NCORES)), trace=trace)
    LAST_EXEC_TIME_NS = res.exec_time_ns
    _CACHE["last_res"] = res

    out = np.empty((B, N, C), dtype=np.float32)
    Q = SW // TP   # 128 rows per (rank, strip)
    for core in range(NCORES):
        b, r = divmod(core, TP)
        ys = np.asarray(res.results[core]["y"], dtype=np.float32)
        for s in range(NSTRIPS):
            out[b, s * SW + r * Q:s * SW + (r + 1) * Q, :] = ys[s * Q:(s + 1) * Q]
    return out


# revision 47
# speedup vs baseline: 1.2704x; 1.2704x over previous
"""nn_CustomAttention on 8 Trainium2 NeuronCores.

Full (unsharded) inputs in, full output out. Internally: data-parallel over
batch (2) x tensor-parallel over heads (16 -> 4 per core), ReduceScatter
(sum over the 4 TP ranks) after the output projection.

Math per batch b (reference):
  qkv = concat(q[b], k[b], v[b]) @ W_qkv.T     # dense over all 3C=3072 inputs
  per head: scores = qh kh^T * hd^-0.5, softmax over keys, x = P @ vh
  out = x @ W_proj.T + b_proj

Device kernel (SPMD, one program for all 8 cores; per-core behavior comes
only from the data each core receives). v3:

  - QKV projection in fp8e4 DoubleRow (half cycles/row, 256-deep
    contraction per matmul) with a same-scale 3-term residual expansion
    x@w ~= x8@w8 + x8@dw8 + dx8@w8 (dw8/dx8 are fp8 of the quantization
    residuals at the SAME scale, living in fp8's subnormal range), which
    restores ~bf16 accuracy at 3/4 of bf16's PE cost. W is pre-scaled x32
    on the host so fp8 hits its normal range; the x32*x32 scores scale is
    folded into the softmax exp's input scale, and the x32 on v into the
    denominator ones-column (=32), so downstream stays at true scale.
  - Attention (scores, exp, AV) and output projection in bf16: fp8 there
    fails the 2e-2 gate (softmax weight quantization error dominates;
    HW-verified). Scores S^T (keys on partitions), softmax denominator =
    ones-row of the AV accumulation, normalize via DVE reciprocal +
    GpSimd partition broadcast.
  - x and W resident in SBUF (fp8 pairs, 132KB/partition total) so the
    projection passes can be scheduled freely: k(s0,s1)+q(s0) accumulate
    cx-major pipelined with the DMA stream, then k(s2,s3); v-projection
    chunks woven into strip 0 pair 0 just before the AV that needs them;
    q(s+1) woven into pair 1 of strip s.
  - bf16 ReduceScatter per strip (the collective has a ~15us constant
    cost; 4 equal strips, the last is the tail), y bf16 (host converts).
"""
import numpy as np
import ml_dtypes

import concourse.bass as bass
import concourse.mybir as mybir
import concourse.tile as tile
from concourse import bacc, bass_utils

B, N, C, H, HD = 2, 2048, 1024, 16, 64
HPC = 4          # heads per core
TP = 4           # tensor-parallel group size
NCORES = 8
SW = 512         # n-strip width
NSTRIPS = N // SW
NJC = N // 128   # key chunks
NC2 = 12         # DoubleRow contraction chunks (3C / 256)
SCALE = HD ** -0.5
F32 = mybir.dt.float32
BF16 = mybir.dt.bfloat16
F8 = mybir.dt.float8e4
DR = mybir.MatmulPerfMode.DoubleRow
ExpF = mybir.ActivationFunctionType.Exp
QKV_FP8 = True    # fp8-DR 3-term residual qkv vs plain bf16 qkv
W_SCALE = 32.0 if QKV_FP8 else 1.0   # host W_qkv scale for fp8 normal range
EXP_BIAS = -3.0   # exp(s-3): softmax-invariant shift, keeps exp moderate

_CACHE = {}
LAST_EXEC_TIME_NS = None


def build_nc(reps=1):
    # reps>1 repeats the whole body (incl. weight DMA) for steady-state
    # benching; the graded path always uses reps=1.
    nc = bacc.Bacc("TRN2", target_bir_lowering=False, debug=False,
                   num_devices=NCORES)
    if QKV_FP8:
        # DoubleRow layouts: chunk c, partition p, tile t -> concat-feature
        # c*256 + t*128 + p
        x8a = nc.dram_tensor("x8a", [NC2, 128, 2, N], F8,
                             kind="ExternalInput").ap()
        dx8a = nc.dram_tensor("dx8a", [NC2, 128, 2, N], F8,
                              kind="ExternalInput").ap()
        w8a = nc.dram_tensor("w8a", [NC2, 128, 2, 768], F8,
                             kind="ExternalInput").ap()
        dw8a = nc.dram_tensor("dw8a", [NC2, 128, 2, 768], F8,
                              kind="ExternalInput").ap()
    else:
        # bf16: chunk c covers concat-features [c*128, (c+1)*128)
        xba = nc.dram_tensor("xba", [2 * NC2, 128, N], BF16,
                             kind="ExternalInput").ap()
        wba = nc.dram_tensor("wba", [2 * NC2, 128, 768], BF16,
                             kind="ExternalInput").ap()
    wproj = nc.dram_tensor("wproj", [2, 128, C], BF16, kind="ExternalInput").ap()
    bias = nc.dram_tensor("bias", [1, C], F32, kind="ExternalInput").ap()
    # y rows: quarter q (128 rows) = this rank's chunk of the RS over
    # n-strip q ([q*512, (q+1)*512)); bf16 — host converts back to f32
    y = nc.dram_tensor("y", [N // TP, C], BF16, kind="ExternalOutput").ap()

    with tile.TileContext(nc) as tc:
      for rep in range(reps):
        with tc.tile_pool(name=f"singles{rep}", bufs=1) as singles, \
             tc.tile_pool(name=f"dram{rep}", bufs=1, space="DRAM") as dram:
            x_sb, dx_sb, w_sb, dw_sb = [], [], [], []
            if QKV_FP8:
                for cx in range(NC2):
                    for lst, srct, nm in ((w_sb, w8a, "w"), (dw_sb, dw8a, "dw"),
                                          (x_sb, x8a, "x"), (dx_sb, dx8a, "dx")):
                        shape = ([128, 2, 768] if nm in ("w", "dw")
                                 else [128, 2, N])
                        t = singles.tile(shape, F8, name=f"{nm}{cx}",
                                         tag=f"{nm}{cx}")
                        nc.sync.dma_start(t[:], srct[cx])
                        lst.append(t)
            else:
                for cx in range(2 * NC2):
                    wt = singles.tile([128, 768], BF16, name=f"w{cx}",
                                      tag=f"w{cx}")
                    nc.sync.dma_start(wt[:], wba[cx])
                    w_sb.append(wt)
                    xt = singles.tile([128, N], BF16, name=f"x{cx}",
                                      tag=f"x{cx}")
                    nc.sync.dma_start(xt[:], xba[cx])
                    x_sb.append(xt)
            wp_tiles = []
            for co in range(2):
                wpt = singles.tile([128, C], BF16, name=f"wp{co}", tag=f"wp{co}")
                nc.sync.dma_start(wpt[:], wproj[co])
                wp_tiles.append(wpt)
            bias_sb = singles.tile([1, C], F32, name="bias_sb")
            nc.sync.dma_start(bias_sb[:], bias)
            bias_bc = singles.tile([128, C], F32, name="bias_bc")
            nc.gpsimd.partition_broadcast(bias_bc[:], bias_sb[:])

            # q,k head-transposed projections (x32 scale): rows = 2 heads
            # x 64d; fc 0,1 = q heads (0,1),(2,3); fc 2,3 = k heads
            qk_sb = singles.tile([128, 4, N], BF16, name="qk_sb")
            # v key-major bf16 (x32) + ones(=32) column per head
            v_sb = singles.tile([128, NJC, HPC, 65], BF16, name="v_sb")
            ones1 = singles.tile([128, 1], F32, name="ones1")
            nc.vector.memset(ones1[:], W_SCALE)
            ebias = singles.tile([128, 1], F32, name="ebias")
            nc.vector.memset(ebias[:], EXP_BIAS)
            nc.vector.tensor_copy(
                v_sb[:, :, :, 64],
                ones1[:, :, None].to_broadcast([128, NJC, HPC]))
            # normalized attention out, feature-major: [ci, co, n]
            oT_sb = singles.tile([128, 2, N], BF16, name="oT_sb")

            cc_in = dram.tile([N, C], BF16, name="cc_in")
            # NOTE: Shared addr_space is only allowed for AllGather/AllReduce
            cc_out = [dram.tile([SW // TP, C], BF16, name=f"cc_out{i}")
                      for i in range(NSTRIPS)]

            # the 3 residual terms: (stationary-w, moving-x) operand pairs
            TERMS = ((w_sb, x_sb), (dw_sb, x_sb), (w_sb, dx_sb))

            # Static PSUM map (8 banks): "s" [128,1024] bufs=2 -> 4 banks
            # (scores, also phase-A accumulators via half-slices), po0/po1
            # -> 2 banks, "aux" [128,512] bufs=2 -> 2 banks (shared in
            # sequence by phase-A, v-proj, q-weave and proj tiles).
            with tc.tile_pool(name="ep", bufs=4) as ep, \
                 tc.tile_pool(name="smp", bufs=2) as smp, \
                 tc.tile_pool(name="outp", bufs=4) as outp, \
                 tc.tile_pool(name="ps_b", bufs=2, space="PSUM") as ps_b:

                def qk_psums(n):
                    """n concurrent accumulation slices [128, SW]: two "s"
                    tiles (split in half) + two aux tiles + (phase A only,
                    while attention hasn't started) the two po banks."""
                    assert n <= 8
                    out = []
                    for i in range(min(2, (n + 1) // 2)):
                        t = ps_b.tile([128, 1024], F32, tag="s",
                                      name=f"pbig{i}", bufs=2)
                        out.append(t[:, 0:SW])
                        out.append(t[:, SW:2 * SW])
                    while len(out) < min(n, 6):
                        out.append(ps_b.tile([128, SW], F32, tag="aux",
                                             name="paux", bufs=2)[:])
                    for par in range(n - 6):
                        out.append(ps_b.tile([128, SW], F32, tag=f"po{par}",
                                             name=f"pqk_po{par}", bufs=1)[:])
                    return out[:n]

                def qk_matmul(pt, fc, s, cx, term):
                    """one matmul: the (cx, term) contribution to the
                    [fc-feature-block x strip-s] accumulator"""
                    if QKV_FP8:
                        wop, xop = TERMS[term]
                        nc.tensor.matmul(
                            pt,
                            wop[cx][:, :, fc * 128:(fc + 1) * 128],
                            xop[cx][:, :, s * SW:(s + 1) * SW],
                            start=(cx == 0 and term == 0),
                            stop=(cx == NC2 - 1 and term == 2),
                            perf_mode=DR)
                    else:
                        nc.tensor.matmul(
                            pt,
                            w_sb[cx][:, fc * 128:(fc + 1) * 128],
                            x_sb[cx][:, s * SW:(s + 1) * SW],
                            start=(cx == 0 and term == 0),
                            stop=(cx == 2 * NC2 - 1 and term == 0))

                def qk_pass(groups):
                    """groups: list of (fc, strip). One PSUM slice each,
                    accumulated cx-major so it pipelines with the x DMA."""
                    pts = qk_psums(len(groups))
                    if QKV_FP8:
                        cxterms = [(cx, t) for cx in range(NC2)
                                   for t in range(3)]
                    else:
                        cxterms = [(cx, 0) for cx in range(2 * NC2)]
                    for cx, term in cxterms:
                        for g, (fc, s) in enumerate(groups):
                            qk_matmul(pts[g], fc, s, cx, term)
                    for g, (fc, s) in enumerate(groups):
                        nc.vector.tensor_copy(
                            qk_sb[:, fc, s * SW:(s + 1) * SW], pts[g])

                def make_q_weave(s):
                    """q-projection for strip s as weave units, 2 fc groups
                    x (12 cx x 3 terms), holding both aux buffers."""
                    pts = [ps_b.tile([128, SW], F32, tag="aux",
                                     name=f"pq{i}", bufs=2)
                           for i in range(2)]

                    def unit(cx, term):
                        def emit():
                            for fc in range(2):
                                qk_matmul(pts[fc][:], fc, s, cx, term)
                            if (cx == NC2 - 1 and term == 2) if QKV_FP8 \
                                    else (cx == 2 * NC2 - 1):
                                for fc in range(2):
                                    nc.vector.tensor_copy(
                                        qk_sb[:, fc, s * SW:(s + 1) * SW],
                                        pts[fc][:])
                        return emit
                    if QKV_FP8:
                        return [unit(cx, term)
                                for cx in range(NC2) for term in range(3)]
                    return [unit(cx, 0) for cx in range(2 * NC2)]

                def make_v_weave():
                    """v-projection chunks: chunk c covers tokens (=keys)
                    [c*128,(c+1)*128). Emitted before AV(jc=c) of strip 0."""
                    def unit(c):
                        def emit():
                            pv = ps_b.tile([128, SW], F32, tag="aux",
                                           name="pv", bufs=2)[:, 0:256]
                            if QKV_FP8:
                                for cx in range(NC2):
                                    for term in range(3):
                                        wop, xop = TERMS[term]
                                        nc.tensor.matmul(
                                            pv,
                                            xop[cx][:, :,
                                                    c * 128:(c + 1) * 128],
                                            wop[cx][:, :, 512:768],
                                            start=(cx == 0 and term == 0),
                                            stop=(cx == NC2 - 1 and term == 2),
                                            perf_mode=DR)
                            else:
                                for cx in range(2 * NC2):
                                    nc.tensor.matmul(
                                        pv,
                                        x_sb[cx][:, c * 128:(c + 1) * 128],
                                        w_sb[cx][:, 512:768],
                                        start=(cx == 0),
                                        stop=(cx == 2 * NC2 - 1))
                            nc.vector.tensor_copy(
                                v_sb[:, c, :, 0:64],
                                pv.rearrange("p (h d) -> p h d", h=HPC))
                        return emit
                    return [unit(c) for c in range(NJC)]

                def make_proj_weave(s):
                    """output projection + bias + RS for strip s as weave
                    units (one per [nch, mh] output tile, then the
                    collective+y-DMA as the final unit)."""
                    def unit(nch, mh):
                        def emit():
                            pp = ps_b.tile([128, SW], F32, tag="aux",
                                           name="pp", bufs=2)
                            for co in range(2):
                                nc.tensor.matmul(
                                    pp[:],
                                    oT_sb[:, co, nch * 128:(nch + 1) * 128],
                                    wp_tiles[co][:, mh * SW:(mh + 1) * SW],
                                    start=(co == 0), stop=(co == 1))
                            ot = outp.tile([128, SW], BF16, tag="ot",
                                           name="ot")
                            nc.vector.tensor_add(
                                ot[:], pp[:],
                                bias_bc[:, mh * SW:(mh + 1) * SW])
                            nc.sync.dma_start(
                                cc_in[nch * 128:(nch + 1) * 128,
                                      mh * SW:(mh + 1) * SW], ot[:])
                        return emit

                    def fin():
                        nc.gpsimd.collective_compute(
                            "ReduceScatter", mybir.AluOpType.add,
                            replica_groups=[[0, 1, 2, 3], [4, 5, 6, 7]],
                            ins=[cc_in[s * SW:(s + 1) * SW, :].opt()],
                            outs=[cc_out[s][:].opt()])
                        nc.sync.dma_start(y[s * 128:(s + 1) * 128, :],
                                          cc_out[s][:])
                    return [unit(nch, mh)
                            for nch in range(4 * s, 4 * (s + 1))
                            for mh in range(2)] + [fin]

                def proj_quarter(s):
                    for u in make_proj_weave(s):
                        u()

                def attention_pair(s, p, weave, weave_first=False):
                    """scores+exp+AV for head pair (2p, 2p+1) of strip s,
                    interleaving `weave` closures into the jc loop."""
                    po = [ps_b.tile([65, SW], F32, tag=f"po{par}",
                                    name=f"po{par}", bufs=1)
                          for par in range(2)]
                    wv = list(weave)
                    for jc in range(NJC):
                        ps2 = ps_b.tile([128, 1024], F32, tag="s",
                                        name="ps2", bufs=2)
                        for par in range(2):
                            hp = par * 64
                            nc.tensor.matmul(
                                ps2[:, par * SW:(par + 1) * SW],
                                qk_sb[hp:hp + 64, 2 + p,
                                      jc * 128:(jc + 1) * 128],
                                qk_sb[hp:hp + 64, p,
                                      s * SW:(s + 1) * SW],
                                start=True, stop=True)
                        et = ep.tile([128, 1024], BF16, tag="e", name="et")
                        # scores psum is (32q)x(32k): exp(psum/1024 - 3)
                        nc.scalar.activation(
                            out=et[:], in_=ps2[:], func=ExpF,
                            bias=ebias[:], scale=1.0 / (W_SCALE * W_SCALE))
                        # weave: spread the units evenly over the jc loop
                        take = len(weave) * (jc + 1) // NJC - \
                            len(weave) * jc // NJC
                        if weave_first:
                            for _ in range(take):
                                if wv:
                                    wv.pop(0)()
                        for par in range(2):
                            h = 2 * p + par
                            nc.tensor.matmul(
                                po[par][:], v_sb[:, jc, h, :],
                                et[:, par * SW:(par + 1) * SW],
                                start=(jc == 0), stop=(jc == NJC - 1))
                        if not weave_first:
                            for _ in range(take):
                                if wv:
                                    wv.pop(0)()
                    while wv:
                        wv.pop(0)()
                    for par in range(2):
                        hp = par * 64
                        recip = smp.tile([1, SW], F32, tag=f"recip{par}",
                                         name="recip")
                        nc.vector.reciprocal(recip[:], po[par][64:65, :])
                        bc = smp.tile([64, SW], F32, tag=f"bc{par}",
                                      name="bc")
                        nc.gpsimd.partition_broadcast(bc[:], recip[:])
                        nc.vector.tensor_mul(
                            oT_sb[hp:hp + 64, p, s * SW:(s + 1) * SW],
                            po[par][0:64, :], bc[:])

                # phase A: k(s0..s2) + q(s0) pipelined with the DMA
                # stream (8 groups, borrowing the idle po banks), then the
                # short k(s3) remainder
                qk_pass([(2, 0), (3, 0), (2, 1), (3, 1), (0, 0), (1, 0),
                         (2, 2), (3, 2)])
                qk_pass([(2, 3), (3, 3)])

                v_weave = make_v_weave()
                for s in range(NSTRIPS):
                    if s == 0:
                        attention_pair(s, 0, v_weave, weave_first=True)
                    else:
                        attention_pair(s, 0, [])
                    q_weave = make_q_weave(s + 1) if s + 1 < NSTRIPS else []
                    attention_pair(s, 1, q_weave)
                    proj_quarter(s)
    nc.compile()
    return nc


def make_in_maps(q, k, v, W_qkv, W_proj, b_proj, **_):
    bf = lambda a: np.ascontiguousarray(a).astype(ml_dtypes.bfloat16)
    F8NP = ml_dtypes.float8_e4m3

    def dr(a, cols):
        # [3072, cols] -> [NC2, 128, 2, cols]: (c,p,t) = feature c*256+t*128+p
        return np.ascontiguousarray(
            a.reshape(NC2, 2, 128, cols).transpose(0, 2, 1, 3))

    in_maps = []
    for core in range(NCORES):
        b, r = divmod(core, TP)
        lo, hi = r * HPC * HD, (r + 1) * HPC * HD    # this core's 256 features
        wq = W_qkv[lo:hi, :] * np.float32(SCALE)
        wk = W_qkv[C + lo:C + hi, :]
        wv = W_qkv[2 * C + lo:2 * C + hi, :]
        wsel = np.concatenate([wq, wk, wv], axis=0)       # [768, 3072]
        wqkvT = np.ascontiguousarray(wsel.T) * np.float32(W_SCALE)  # [3072,768]
        wprojT = np.ascontiguousarray(W_proj[:, lo:hi].T)  # [256, 1024]
        bias = b_proj if r == 0 else np.zeros_like(b_proj)
        xcat = np.concatenate(
            [np.ascontiguousarray(q[b].T), np.ascontiguousarray(k[b].T),
             np.ascontiguousarray(v[b].T)], axis=0)       # [3072, N]
        m = {
            "wproj": bf(wprojT.reshape(2, 128, C)),
            "bias": np.ascontiguousarray(bias[None, :], dtype=np.float32),
        }
        if QKV_FP8:
            w8 = wqkvT.astype(F8NP)
            dw8 = (wqkvT - w8.astype(np.float32)).astype(F8NP)
            x8 = xcat.astype(F8NP)
            dx8 = (xcat - x8.astype(np.float32)).astype(F8NP)
            m.update(x8a=dr(x8, N), dx8a=dr(dx8, N),
                     w8a=dr(w8, 768), dw8a=dr(dw8, 768))
        else:
            m.update(xba=bf(xcat.reshape(2 * NC2, 128, N)),
                     wba=bf(wqkvT.reshape(2 * NC2, 128, 768)))
        in_maps.append(m)
    return in_maps


def kernel(q, k, v, W_qkv, W_proj, b_proj, trace=False):
    global LAST_EXEC_TIME_NS
    q = np.asarray(q, dtype=np.float32)
    k = np.asarray(k, dtype=np.float32)
    v = np.asarray(v, dtype=np.float32)
    W_qkv = np.asarray(W_qkv, dtype=np.float32)
    W_proj = np.asarray(W_proj, dtype=np.float32)
    b_proj = np.asarray(b_proj, dtype=np.float32)

    if "nc" not in _CACHE:
        _CACHE["nc"] = build_nc()
    nc = _CACHE["nc"]
    in_maps = make_in_maps(q, k, v, W_qkv, W_proj, b_proj)
    res = bass_utils.run_bass_kernel_spmd(
        nc, in_maps, core_ids=list(range(NCORES)), trace=trace)
    LAST_EXEC_TIME_NS = res.exec_time_ns
    _CACHE["last_res"] = res

    out = np.empty((B, N, C), dtype=np.float32)
    Q = SW // TP   # 128 rows per (rank, strip)
    for core in range(NCORES):
        b, r = divmod(core, TP)
        ys = np.asarray(res.results[core]["y"], dtype=np.float32)
        for s in range(NSTRIPS):
            out[b, s * SW + r * Q:s * SW + (r + 1) * Q, :] = ys[s * Q:(s + 1) * Q]
    return out
